# revision 30
# baseline (speedup 1.0000x reference)
"""Class-conditional label-smoothing cross-entropy loss on 8 Trainium2 cores.

Reference math (C=1000 classes, B=65536 samples, smoothing s=0.1):
    A = softmax(class_avg, axis=-1)                         # [C, C]
    S[t, j] = s * (1 - A[t, j]) / (1 - A[t, t])  (j != t);  S[t, t] = 1 - s
    R[t]    = sum_j S[t, j]
    loss_i  = lse_i * R[t_i] - S[t_i] . x_i,   lse_i = log(sum_j exp(x_ij))
    out     = mean_i loss_i

The loss is a mean over samples, so sample order is free. The host sorts
the WHOLE batch by target class and cuts it into 8 contiguous shards, so
each core sees only ~126 distinct classes with ~65 samples each. Each
core's 128x64 slot grid (sample j*128+p -> partition p, column j) is then
packed into:
  - 7 "pure" bands of 8 columns: each (partition, band) cell holds 8
    samples of ONE class, so one 128-row dma_gather serves 8 columns
    (class-run packing, host-planned).
  - 1 cleanup band (columns 56-63) for the leftover <8-sample class
    remnants and overflow runs, gathered per-sample (1024 rows, one
    dma_gather), class-sorted for HBM row-buffer locality.
This cuts per-core gather traffic from 8.39 MB (64 x 128 rows) to 1.9 MB
(7 x 128 + 1024 rows).

Each core:
  1. builds the smoothing table in its DRAM once:
     tab[t] = [S[t, :] as fp8e4 (1000 B) | R[t] as f32 bit-packed into 4
     fp8 slots | zero pad to 1024 B]  (fp8 quarters the per-sample gather
     traffic; since E[x]=0 the S quantization noise is mean-zero in the
     final scalar, and R stays exact f32 via the bit-pack)
  2. gathers use dma_gather (InstDMAGatherAnt, mlp library): ONE
     instruction gathers up to 1024 rows (vs one indirect_dma_start per
     column, whose ~1 us fixed SWDGE descriptor-emission cost serialized on
     the Pool engine and dominated the old schedule). dma_gather's native
     output layout dst[i%128, i//128, :] = tab[idx_i] matches the
     column-major slot layout exactly.
  3. per column: ACT exp with accumulate -> sumexp, one fused DVE multiply
     with accumulate -> dot; R (bit-packed f32) is copied once per band
     (free-dim broadcast) or per cleanup column.
  4. tail: lse = ln(sumexp), loss = R*lse - dot, one [128, 64] store.
Host sums the 8 partial grids in f64 and divides by B.
"""

import contextlib

import numpy as np

import concourse.bass as bass
import concourse.tile as tile
from concourse import bacc, library_config, mybir
from concourse.bass_utils import run_bass_kernel_spmd

B = 65536
C = 1000
NCORES = 8
BLOC = B // NCORES          # 8192 samples per core
P = 128
NT = BLOC // P              # 64 sample columns per core
TABW = 1024                 # table row: 1000 fp8 S + f32 R (4 slots) + pad
SM = 0.1
BCOL = 8                    # columns per band
PB = 7                      # pure bands (one gather per band), uniform case

_CACHE = {}
UNROLL = 4                  # main-loop passes per hardware-loop iteration


def n_passes(reps):
    """Main-loop passes executed by build_program(reps)."""
    return 1 if reps == 1 else reps * UNROLL


def n_clean(pb):
    return (NT - pb * BCOL) * P


def n_idx(pb):
    return pb * P + n_clean(pb)


def idx_w(pb):
    return n_idx(pb) // 16


def build_program(reps=1, abl=(), x_chunk=4, xs_bufs=5, gs_bufs=6, scr_bufs=6, pb=PB):
    # abl: timing-ablation switches ("gather" | "x" | "act" | "dve"), each
    # drops that component from the main loop (breaks numerics, timing only).
    # x_chunk: sample columns per x DMA (2 -> 1 MiB transfers).
    # reps>1 wraps the main loop in a tc.For_i HARDWARE loop (same data every
    # pass) for slope-timing in test.py: device time scales with reps while
    # compile time and dispatch overhead do not. The loop body holds UNROLL
    # unrolled passes so the loop's per-iteration all-engine barrier (a
    # pipeline drain the production reps=1 program does not have) is paid
    # every 4th pass only; total passes = n_passes(reps).
    f32 = mybir.dt.float32
    bf16 = mybir.dt.bfloat16
    i16 = mybir.dt.int16
    Alu = mybir.AluOpType
    Act = mybir.ActivationFunctionType
    tdt = mybir.dt.float8e4
    rslots = 4  # R occupies 4 fp8 slots right after the C S-entries

    IDXW = idx_w(pb)
    nc = bacc.Bacc("TRN2", target_bir_lowering=False, debug=False)
    x_ap = nc.dram_tensor("x", [BLOC, C], f32, kind="ExternalInput").ap()
    ca_ap = nc.dram_tensor("ca", [C, C], f32, kind="ExternalInput").ap()
    tg_ap = nc.dram_tensor("tgw", [P, IDXW], i16, kind="ExternalInput").ap()
    out_ap = nc.dram_tensor("out", [P, NT], f32, kind="ExternalOutput").ap()
    tab_ap = nc.dram_tensor("tab", [C, TABW], tdt).ap()

    with tile.TileContext(nc) as tc:
        nc.gpsimd.load_library(library_config.mlp)
        with (
            tc.tile_pool(name="tabp", bufs=2) as tabp,
            tc.tile_pool(name="small", bufs=2) as small,
            tc.tile_pool(name="xs", bufs=xs_bufs) as xs,
            tc.tile_pool(name="gbp", bufs=gs_bufs) as gbp,
            tc.tile_pool(name="gcp", bufs=2) as gcp,
            tc.tile_pool(name="scr", bufs=scr_bufs) as scr,
            tc.tile_pool(name="cols", bufs=1) as cols,
        ):
            # wrapped+replicated target indices for dma_gather
            idx = cols.tile([P, IDXW], i16)
            nc.sync.dma_start(idx[:], tg_ap[:, :])

            # ---- smoothing table -------------------------------------------
            for k in range((C + P - 1) // P):
                r0 = k * P
                pr = min(r0 + P, C) - r0
                cat = tabp.tile([P, C], f32, tag="cat")
                nc.sync.dma_start(cat[:pr], ca_ap[r0 : r0 + pr, :])
                e = tabp.tile([P, C], f32, tag="e")
                sume = small.tile([P, 1], f32, tag="sume")
                nc.scalar.activation(e[:pr], cat[:pr], Act.Exp, accum_out=sume[:pr])
                # diagonal e[t, t] via affine mask + row reduce
                msk = tabp.tile([P, C], f32, tag="msk")
                nc.gpsimd.affine_select(
                    out=msk[:pr], in_=e[:pr], compare_op=Alu.is_equal, fill=0.0,
                    base=-r0, channel_multiplier=-1, pattern=[[1, C]],
                )
                ett = small.tile([P, 1], f32, tag="ett")
                nc.vector.tensor_reduce(
                    out=ett[:pr], in_=msk[:pr], axis=mybir.AxisListType.X, op=Alu.add
                )
                den = small.tile([P, 1], f32, tag="den")
                nc.vector.tensor_tensor(
                    out=den[:pr], in0=sume[:pr], in1=ett[:pr], op=Alu.subtract
                )
                rec = small.tile([P, 1], f32, tag="rec")
                nc.vector.reciprocal(rec[:pr], den[:pr])
                negw = small.tile([P, 1], f32, tag="negw")
                nc.vector.tensor_scalar_mul(negw[:pr], rec[:pr], -SM)
                # S_pre[t, j] = (e - sume) * (-s / den); its diagonal equals s,
                # and sum_j S_pre = R - (1 - 2s)
                spre = tabp.tile([P, C], f32, tag="spre")
                rpre = small.tile([P, 1], f32, tag="rpre")
                nc.vector.scalar_tensor_tensor(
                    out=spre[:pr], in0=e[:pr], scalar=sume[:pr],
                    in1=negw[:pr].to_broadcast([pr, C]),
                    op0=Alu.subtract, op1=Alu.mult, accum_out=rpre[:pr],
                )
                sb = tabp.tile([P, TABW], tdt, tag="sb")
                nc.gpsimd.affine_select(
                    out=sb[:pr, 0:C], in_=spre[:pr], compare_op=Alu.not_equal,
                    fill=1.0 - SM, base=-r0, channel_multiplier=-1, pattern=[[1, C]],
                )
                # R as raw f32 bits in the 4 fp8 slots after the S entries
                rt = small.tile([P, 1], f32, tag="rt")
                nc.vector.tensor_scalar_add(rt[:pr], rpre[:pr], 1.0 - 2 * SM)
                rv = sb[:pr, C : C + rslots].bitcast(f32)
                nc.vector.tensor_copy(out=rv[:, 0:1], in_=rt[:pr])
                nc.vector.memset(sb[:pr, C + rslots : TABW], 0.0)
                nc.sync.dma_start(tab_ap[r0 : r0 + pr, :], sb[:pr])

            # ---- main loop -------------------------------------------------
            # x viewed column-major: sample j*128 + p -> partition p, col j,
            # so an x_chunk load is one DMA over a contiguous DRAM range
            x_r = x_ap.rearrange("(c p) d -> p c d", p=P)
            se_cols = cols.tile([P, NT], f32)
            dot_cols = cols.tile([P, NT], f32)
            r_cols = cols.tile([P, NT], f32)
            if abl:
                nc.vector.memset(se_cols[:], 1.0)
                nc.vector.memset(dot_cols[:], 1.0)
                nc.vector.memset(r_cols[:], 1.0)
            xt0 = gt0 = None
            if "x" in abl:
                xt0 = cols.tile([P, C], f32)
                nc.sync.dma_start(xt0[:], x_r[:, 0, :])
            if "gather" in abl:
                gt0 = cols.tile([P, 1, TABW], tdt)
                nc.vector.memset(gt0[:, 0, :], 0.25)
            xbig = gband = gclean = None
            loop = tc.For_i(0, reps) if reps > 1 else contextlib.nullcontext()
            with loop:
              for j in range(NT * (UNROLL if reps > 1 else 1)):
                j = j % NT
                if "x" in abl:
                    xt = xt0
                else:
                    if j % x_chunk == 0:
                        xbig = xs.tile([P, x_chunk, C], f32)
                        nc.sync.dma_start(xbig[:], x_r[:, j : j + x_chunk, :])
                    xt = xbig[:, j % x_chunk, :]
                in_pure = j < pb * BCOL
                new_band = in_pure and j % BCOL == 0
                if "gather" in abl:
                    gt = gt0[:, 0, :]
                elif in_pure:
                    b = j // BCOL
                    if new_band:
                        # one row per (partition, band) cell, reused 8 cols
                        gband = gbp.tile([P, 1, TABW], tdt, tag="gband")
                        nc.gpsimd.dma_gather(
                            gband[:], tab_ap[:],
                            idx[:, b * (P // 16) : (b + 1) * (P // 16)],
                            P, P, TABW,
                        )
                    gt = gband[:, 0, :]
                else:
                    jc = j - pb * BCOL
                    if jc % BCOL == 0:
                        # per-sample gather, one 8-column chunk at a time
                        gclean = gcp.tile([P, BCOL, TABW], tdt, tag="gc")
                        c0 = pb * (P // 16) + jc * (P // 16)
                        nc.gpsimd.dma_gather(
                            gclean[:], tab_ap[:],
                            idx[:, c0 : c0 + BCOL * (P // 16)],
                            BCOL * P, BCOL * P, TABW,
                        )
                    gt = gclean[:, jc % BCOL, :]
                if "act" not in abl:
                    es = scr.tile([P, C], bf16, tag="es")
                    nc.scalar.activation(
                        es[:], xt[:], Act.Exp, accum_out=se_cols[:, j : j + 1]
                    )
                if "dve" not in abl:
                    ps = scr.tile([P, C], f32, tag="ps")
                    nc.vector.scalar_tensor_tensor(
                        out=ps[:], in0=xt[:], scalar=1.0, in1=gt[:, 0:C],
                        op0=Alu.mult, op1=Alu.mult, accum_out=dot_cols[:, j : j + 1],
                    )
                    grv = gt[:, C : C + rslots].bitcast(f32)
                    if new_band:
                        nc.vector.tensor_copy(
                            out=r_cols[:, j : j + BCOL],
                            in_=grv[:, 0:1].to_broadcast([P, BCOL]),
                        )
                    elif not in_pure:
                        nc.vector.tensor_copy(
                            out=r_cols[:, j : j + 1], in_=grv[:, 0:1]
                        )

            # ---- tail ------------------------------------------------------
            lse = cols.tile([P, NT], f32)
            nc.scalar.activation(lse[:], se_cols[:], Act.Ln)
            t1 = cols.tile([P, NT], f32)
            nc.vector.tensor_mul(t1[:], r_cols[:], lse[:])
            loss = cols.tile([P, NT], f32)
            nc.vector.tensor_tensor(
                out=loss[:], in0=t1[:], in1=dot_cols[:], op=Alu.subtract
            )
            nc.sync.dma_start(out_ap[:], loss[:])

    nc.compile()
    nc.finalize()
    return nc


def get_program(pb=PB):
    key = ("nc", pb)
    if key not in _CACHE:
        _CACHE[key] = build_program(pb=pb)
    return _CACHE[key]


def wrap_idx(stream, pb=PB):
    """[NIDX] int -> [128, NIDX/16] int16 wrapped (pos i -> [i%16, i//16])
    and replicated across the 8 partition groups of 16 (one per Q7 core)."""
    w = stream.astype(np.int16).reshape(idx_w(pb), 16).T  # [16, IDXW]
    return np.ascontiguousarray(np.tile(w, (8, 1)))  # [128, IDXW]


def max_pure_bands(ts):
    """Max pure bands a shard supports: full 8-run count // 128, capped."""
    cnt = np.bincount(ts, minlength=C)
    return int(min(PB, (cnt // BCOL).sum() // P))


def plan_shard(ts, pb=PB):
    """Pack one class-sorted shard into the slot grid.

    Returns (order, idx_stream): order[slot_linear] = shard sample row for
    device slot (p, j) with slot_linear = j*128 + p; idx_stream[n_idx(pb)] =
    the gather-index stream (pb*128 band cells then cleanup samples).
    """
    n = ts.shape[0]
    assert n == BLOC
    srt = np.argsort(ts, kind="stable")
    tss = ts[srt]
    # class group boundaries in the sorted shard
    bounds = np.flatnonzero(np.r_[True, tss[1:] != tss[:-1], True])
    runs = []          # full 8-sample runs: (class, 8 sample rows)
    cleanup = []       # leftover sample rows (class-sorted order)
    ncells = pb * P
    for g0, g1 in zip(bounds[:-1], bounds[1:]):
        cls = int(tss[g0])
        k = g0
        while k + BCOL <= g1 and len(runs) < ncells:
            runs.append((cls, srt[k : k + BCOL]))
            k += BCOL
        if k < g1:
            cleanup.append((cls, srt[k:g1]))
    assert len(runs) == ncells, (
        f"class-run packing infeasible: {len(runs)} full runs < {ncells}"
    )
    order = np.empty(n, dtype=np.int64)
    idx_stream = np.empty(n_idx(pb), dtype=np.int64)
    if ncells:
        # run r -> band b = r // 128, partition p = r % 128, cols b*8..b*8+7
        rcls = np.array([c for c, _ in runs])
        rsmp = np.stack([s for _, s in runs])          # [ncells, 8]
        r = np.arange(ncells)
        lin = ((r[:, None] // P) * BCOL + np.arange(BCOL)[None, :]) * P + (
            r[:, None] % P
        )
        order[lin.ravel()] = rsmp.ravel()
        idx_stream[:ncells] = rcls
    # cleanup sample m -> column pb*8 + m//128, partition m%128
    csmp = np.concatenate([s for _, s in cleanup]) if cleanup else np.empty(0, int)
    ccls = (
        np.concatenate([np.full(len(s), c) for c, s in cleanup])
        if cleanup
        else np.empty(0, int)
    )
    assert csmp.shape[0] == n_clean(pb), csmp.shape
    order[ncells * BCOL :] = csmp
    idx_stream[ncells:] = ccls
    return order, idx_stream


def make_in_maps(x, class_avg, target, pb=PB):
    x = np.ascontiguousarray(np.asarray(x, dtype=np.float32))
    ca = np.ascontiguousarray(np.asarray(class_avg, dtype=np.float32))
    tg = np.asarray(target).astype(np.int32)
    assert x.shape == (B, C) and ca.shape == (C, C) and tg.shape == (B,)
    # global class sort -> contiguous shards span ~C/8 classes each
    gsort = np.argsort(tg, kind="stable")
    maps = []
    for c in range(NCORES):
        rows = gsort[c * BLOC : (c + 1) * BLOC]
        xs, ts = x[rows], tg[rows]
        order, idx_stream = plan_shard(ts, pb)
        maps.append(
            {
                "x": np.ascontiguousarray(xs[order]),
                "ca": ca,
                "tgw": wrap_idx(idx_stream, pb),
            }
        )
    return maps


def pick_pb(target):
    """Largest pb all shards support (7 for any near-uniform targets)."""
    tg = np.asarray(target).astype(np.int32)
    gsort = np.argsort(tg, kind="stable")
    return min(
        max_pure_bands(tg[gsort[c * BLOC : (c + 1) * BLOC]]) for c in range(NCORES)
    )


def reduce_outputs(results):
    tot = 0.0
    for c in range(NCORES):
        tot += results[c]["out"].astype(np.float64).sum()
    return np.array(tot / B, dtype=np.float32)


def kernel(x, class_avg, target):
    pb = pick_pb(target)
    nc = get_program(pb)
    in_maps = make_in_maps(x, class_avg, target, pb)
    res = run_bass_kernel_spmd(nc, in_maps, list(range(NCORES)))
    return reduce_outputs(res.results)


# revision 34
# speedup vs baseline: 1.3630x; 1.3630x over previous
"""Class-conditional label-smoothing cross-entropy loss on 8 Trainium2 cores.

Reference math (C=1000 classes, B=65536 samples, smoothing s=0.1):
    A = softmax(class_avg, axis=-1)                         # [C, C]
    S[t, j] = s * (1 - A[t, j]) / (1 - A[t, t])  (j != t);  S[t, t] = 1 - s
    R[t]    = sum_j S[t, j]
    loss_i  = lse_i * R[t_i] - S[t_i] . x_i,   lse_i = log(sum_j exp(x_ij))
    out     = mean_i loss_i

The loss is a mean over samples, so sample order is free. The host sorts
the WHOLE batch by target class and cuts it into 8 contiguous shards, so
each core sees only ~126 distinct classes with ~65 samples each. Each
core's 128x64 slot grid (sample j*128+p -> partition p, column j) is then
packed into:
  - 7 "pure" bands of 8 columns: each (partition, band) cell holds 8
    samples of ONE class, so one 128-row dma_gather serves 8 columns
    (class-run packing, host-planned).
  - 1 cleanup band (columns 56-63) for the leftover <8-sample class
    remnants and overflow runs, gathered per-sample (1024 rows, one
    dma_gather), class-sorted for HBM row-buffer locality.
This cuts per-core gather traffic from 8.39 MB (64 x 128 rows) to 1.9 MB
(7 x 128 + 1024 rows).

Each core:
  1. builds the smoothing table in its DRAM once:
     tab[t] = [S[t, :] as fp8e4 (1000 B) | R[t] as f32 bit-packed into 4
     fp8 slots | zero pad to 1024 B]  (fp8 quarters the per-sample gather
     traffic; since E[x]=0 the S quantization noise is mean-zero in the
     final scalar, and R stays exact f32 via the bit-pack)
  2. gathers use dma_gather (InstDMAGatherAnt, mlp library): ONE
     instruction gathers up to 1024 rows (vs one indirect_dma_start per
     column, whose ~1 us fixed SWDGE descriptor-emission cost serialized on
     the Pool engine and dominated the old schedule). dma_gather's native
     output layout dst[i%128, i//128, :] = tab[idx_i] matches the
     column-major slot layout exactly.
  3. per column: ACT exp with accumulate -> sumexp, one fused DVE multiply
     with accumulate -> dot; R (bit-packed f32) is copied once per band
     (free-dim broadcast) or per cleanup column.
  4. tail: lse = ln(sumexp), loss = R*lse - dot, one [128, 64] store.
Host sums the 8 partial grids in f64 and divides by B.
"""

import contextlib

import numpy as np

import concourse.bass as bass
import concourse.tile as tile
from concourse import bacc, library_config, mybir
from concourse.bass_utils import run_bass_kernel_spmd

B = 65536
C = 1000
NCORES = 8
BLOC = B // NCORES          # 8192 samples per core
P = 128
NT = BLOC // P              # 64 sample columns per core
TABW = 1024                 # table row: 1000 fp8 S + f32 R (4 slots) + pad
SM = 0.1
BCOL = 8                    # columns per band
PB = 7                      # pure bands (one gather per band), uniform case

_CACHE = {}
# Main-loop passes per hardware-loop iteration in the reps>1 timing build.
# KEEP AT 1: a 4-pass unrolled body measured 134.7 us/pass vs 111.9 for the
# 1-pass body (HW, congestion-immune looped-program diff) — the larger body
# appears to overflow the NX sequencers' instruction cache, so every
# For_i iteration re-streams it, stealing HBM bandwidth from the x loads.
UNROLL = 1


def n_passes(reps):
    """Main-loop passes executed by build_program(reps)."""
    return 1 if reps == 1 else reps * UNROLL


def n_clean(pb):
    return (NT - pb * BCOL) * P


def n_idx(pb):
    return pb * P + n_clean(pb)


def idx_w(pb):
    return n_idx(pb) // 16


def build_program(
    reps=1, abl=(), x_chunk=4, xs_bufs=5, gs_bufs=6, scr_bufs=6, pb=PB, unroll=UNROLL
):
    # abl: timing-ablation switches ("gather" | "x" | "act" | "dve"), each
    # drops that component from the main loop (breaks numerics, timing only).
    # x_chunk: sample columns per x DMA (2 -> 1 MiB transfers).
    # reps>1 wraps the main loop in a tc.For_i HARDWARE loop (same data every
    # pass) for slope-timing in test.py: device time scales with reps while
    # compile time and dispatch overhead do not. The per-iteration all-engine
    # barrier is a pipeline drain the production reps=1 program does not
    # have, so the slope slightly OVER-estimates the production pass.
    f32 = mybir.dt.float32
    bf16 = mybir.dt.bfloat16
    i16 = mybir.dt.int16
    Alu = mybir.AluOpType
    Act = mybir.ActivationFunctionType
    tdt = mybir.dt.float8e4
    rslots = 4  # R occupies 4 fp8 slots right after the C S-entries

    IDXW = idx_w(pb)
    nc = bacc.Bacc("TRN2", target_bir_lowering=False, debug=False)
    x_ap = nc.dram_tensor("x", [BLOC, C], f32, kind="ExternalInput").ap()
    ca_ap = nc.dram_tensor("ca", [C, C], f32, kind="ExternalInput").ap()
    tg_ap = nc.dram_tensor("tgw", [P, IDXW], i16, kind="ExternalInput").ap()
    out_ap = nc.dram_tensor("out", [P, NT], f32, kind="ExternalOutput").ap()
    tab_ap = nc.dram_tensor("tab", [C, TABW], tdt).ap()

    with tile.TileContext(nc) as tc:
        nc.gpsimd.load_library(library_config.mlp)
        with (
            tc.tile_pool(name="tabp", bufs=2) as tabp,
            tc.tile_pool(name="small", bufs=2) as small,
            tc.tile_pool(name="xs", bufs=xs_bufs) as xs,
            tc.tile_pool(name="gbp", bufs=gs_bufs) as gbp,
            tc.tile_pool(name="gcp", bufs=2) as gcp,
            tc.tile_pool(name="scr", bufs=scr_bufs) as scr,
            tc.tile_pool(name="cols", bufs=1) as cols,
        ):
            # wrapped+replicated target indices for dma_gather
            idx = cols.tile([P, IDXW], i16)
            nc.sync.dma_start(idx[:], tg_ap[:, :])

            # ---- smoothing table -------------------------------------------
            for k in range((C + P - 1) // P):
                r0 = k * P
                pr = min(r0 + P, C) - r0
                cat = tabp.tile([P, C], f32, tag="cat")
                nc.sync.dma_start(cat[:pr], ca_ap[r0 : r0 + pr, :])
                e = tabp.tile([P, C], f32, tag="e")
                sume = small.tile([P, 1], f32, tag="sume")
                nc.scalar.activation(e[:pr], cat[:pr], Act.Exp, accum_out=sume[:pr])
                # diagonal e[t, t] via affine mask + row reduce
                msk = tabp.tile([P, C], f32, tag="msk")
                nc.gpsimd.affine_select(
                    out=msk[:pr], in_=e[:pr], compare_op=Alu.is_equal, fill=0.0,
                    base=-r0, channel_multiplier=-1, pattern=[[1, C]],
                )
                ett = small.tile([P, 1], f32, tag="ett")
                nc.vector.tensor_reduce(
                    out=ett[:pr], in_=msk[:pr], axis=mybir.AxisListType.X, op=Alu.add
                )
                den = small.tile([P, 1], f32, tag="den")
                nc.vector.tensor_tensor(
                    out=den[:pr], in0=sume[:pr], in1=ett[:pr], op=Alu.subtract
                )
                rec = small.tile([P, 1], f32, tag="rec")
                nc.vector.reciprocal(rec[:pr], den[:pr])
                negw = small.tile([P, 1], f32, tag="negw")
                nc.vector.tensor_scalar_mul(negw[:pr], rec[:pr], -SM)
                # S_pre[t, j] = (e - sume) * (-s / den); its diagonal equals s,
                # and sum_j S_pre = R - (1 - 2s)
                spre = tabp.tile([P, C], f32, tag="spre")
                rpre = small.tile([P, 1], f32, tag="rpre")
                nc.vector.scalar_tensor_tensor(
                    out=spre[:pr], in0=e[:pr], scalar=sume[:pr],
                    in1=negw[:pr].to_broadcast([pr, C]),
                    op0=Alu.subtract, op1=Alu.mult, accum_out=rpre[:pr],
                )
                sb = tabp.tile([P, TABW], tdt, tag="sb")
                nc.gpsimd.affine_select(
                    out=sb[:pr, 0:C], in_=spre[:pr], compare_op=Alu.not_equal,
                    fill=1.0 - SM, base=-r0, channel_multiplier=-1, pattern=[[1, C]],
                )
                # R as raw f32 bits in the 4 fp8 slots after the S entries
                rt = small.tile([P, 1], f32, tag="rt")
                nc.vector.tensor_scalar_add(rt[:pr], rpre[:pr], 1.0 - 2 * SM)
                rv = sb[:pr, C : C + rslots].bitcast(f32)
                nc.vector.tensor_copy(out=rv[:, 0:1], in_=rt[:pr])
                nc.vector.memset(sb[:pr, C + rslots : TABW], 0.0)
                nc.sync.dma_start(tab_ap[r0 : r0 + pr, :], sb[:pr])

            # ---- main loop -------------------------------------------------
            # x viewed column-major: sample j*128 + p -> partition p, col j,
            # so an x_chunk load is one DMA over a contiguous DRAM range
            x_r = x_ap.rearrange("(c p) d -> p c d", p=P)
            se_cols = cols.tile([P, NT], f32)
            dot_cols = cols.tile([P, NT], f32)
            r_cols = cols.tile([P, NT], f32)
            if abl:
                nc.vector.memset(se_cols[:], 1.0)
                nc.vector.memset(dot_cols[:], 1.0)
                nc.vector.memset(r_cols[:], 1.0)
            xt0 = gt0 = None
            if "x" in abl:
                xt0 = cols.tile([P, C], f32)
                nc.sync.dma_start(xt0[:], x_r[:, 0, :])
            if "gather" in abl:
                gt0 = cols.tile([P, 1, TABW], tdt)
                nc.vector.memset(gt0[:, 0, :], 0.25)
            xbig = gband = gclean = None
            loop = tc.For_i(0, reps) if reps > 1 else contextlib.nullcontext()
            with loop:
              for j in range(NT * (unroll if reps > 1 else 1)):
                j = j % NT
                if "x" in abl:
                    xt = xt0
                else:
                    if j % x_chunk == 0:
                        xbig = xs.tile([P, x_chunk, C], f32)
                        nc.sync.dma_start(xbig[:], x_r[:, j : j + x_chunk, :])
                    xt = xbig[:, j % x_chunk, :]
                in_pure = j < pb * BCOL
                new_band = in_pure and j % BCOL == 0
                if "gather" in abl:
                    gt = gt0[:, 0, :]
                elif in_pure:
                    b = j // BCOL
                    if new_band:
                        # one row per (partition, band) cell, reused 8 cols
                        gband = gbp.tile([P, 1, TABW], tdt, tag="gband")
                        nc.gpsimd.dma_gather(
                            gband[:], tab_ap[:],
                            idx[:, b * (P // 16) : (b + 1) * (P // 16)],
                            P, P, TABW,
                        )
                    gt = gband[:, 0, :]
                else:
                    jc = j - pb * BCOL
                    if jc % BCOL == 0:
                        # per-sample gather, one 8-column chunk at a time
                        gclean = gcp.tile([P, BCOL, TABW], tdt, tag="gc")
                        c0 = pb * (P // 16) + jc * (P // 16)
                        nc.gpsimd.dma_gather(
                            gclean[:], tab_ap[:],
                            idx[:, c0 : c0 + BCOL * (P // 16)],
                            BCOL * P, BCOL * P, TABW,
                        )
                    gt = gclean[:, jc % BCOL, :]
                if "act" not in abl:
                    es = scr.tile([P, C], bf16, tag="es")
                    nc.scalar.activation(
                        es[:], xt[:], Act.Exp, accum_out=se_cols[:, j : j + 1]
                    )
                if "dve" not in abl:
                    ps = scr.tile([P, C], f32, tag="ps")
                    nc.vector.scalar_tensor_tensor(
                        out=ps[:], in0=xt[:], scalar=1.0, in1=gt[:, 0:C],
                        op0=Alu.mult, op1=Alu.mult, accum_out=dot_cols[:, j : j + 1],
                    )
                    grv = gt[:, C : C + rslots].bitcast(f32)
                    if new_band:
                        nc.vector.tensor_copy(
                            out=r_cols[:, j : j + BCOL],
                            in_=grv[:, 0:1].to_broadcast([P, BCOL]),
                        )
                    elif not in_pure:
                        nc.vector.tensor_copy(
                            out=r_cols[:, j : j + 1], in_=grv[:, 0:1]
                        )

            # ---- tail ------------------------------------------------------
            lse = cols.tile([P, NT], f32)
            nc.scalar.activation(lse[:], se_cols[:], Act.Ln)
            t1 = cols.tile([P, NT], f32)
            nc.vector.tensor_mul(t1[:], r_cols[:], lse[:])
            loss = cols.tile([P, NT], f32)
            nc.vector.tensor_tensor(
                out=loss[:], in0=t1[:], in1=dot_cols[:], op=Alu.subtract
            )
            nc.sync.dma_start(out_ap[:], loss[:])

    nc.compile()
    nc.finalize()
    return nc


def get_program(pb=PB):
    key = ("nc", pb)
    if key not in _CACHE:
        _CACHE[key] = build_program(pb=pb)
    return _CACHE[key]


def wrap_idx(stream, pb=PB):
    """[NIDX] int -> [128, NIDX/16] int16 wrapped (pos i -> [i%16, i//16])
    and replicated across the 8 partition groups of 16 (one per Q7 core)."""
    w = stream.astype(np.int16).reshape(idx_w(pb), 16).T  # [16, IDXW]
    return np.ascontiguousarray(np.tile(w, (8, 1)))  # [128, IDXW]


def max_pure_bands(ts):
    """Max pure bands a shard supports: full 8-run count // 128, capped."""
    cnt = np.bincount(ts, minlength=C)
    return int(min(PB, (cnt // BCOL).sum() // P))


def plan_shard(ts, pb=PB):
    """Pack one class-sorted shard into the slot grid.

    Returns (order, idx_stream): order[slot_linear] = shard sample row for
    device slot (p, j) with slot_linear = j*128 + p; idx_stream[n_idx(pb)] =
    the gather-index stream (pb*128 band cells then cleanup samples).
    """
    n = ts.shape[0]
    assert n == BLOC
    srt = np.argsort(ts, kind="stable")
    tss = ts[srt]
    # class group boundaries in the sorted shard
    bounds = np.flatnonzero(np.r_[True, tss[1:] != tss[:-1], True])
    runs = []          # full 8-sample runs: (class, 8 sample rows)
    cleanup = []       # leftover sample rows (class-sorted order)
    ncells = pb * P
    for g0, g1 in zip(bounds[:-1], bounds[1:]):
        cls = int(tss[g0])
        k = g0
        while k + BCOL <= g1 and len(runs) < ncells:
            runs.append((cls, srt[k : k + BCOL]))
            k += BCOL
        if k < g1:
            cleanup.append((cls, srt[k:g1]))
    assert len(runs) == ncells, (
        f"class-run packing infeasible: {len(runs)} full runs < {ncells}"
    )
    order = np.empty(n, dtype=np.int64)
    idx_stream = np.empty(n_idx(pb), dtype=np.int64)
    if ncells:
        # run r -> band b = r // 128, partition p = r % 128, cols b*8..b*8+7
        rcls = np.array([c for c, _ in runs])
        rsmp = np.stack([s for _, s in runs])          # [ncells, 8]
        r = np.arange(ncells)
        lin = ((r[:, None] // P) * BCOL + np.arange(BCOL)[None, :]) * P + (
            r[:, None] % P
        )
        order[lin.ravel()] = rsmp.ravel()
        idx_stream[:ncells] = rcls
    # cleanup sample m -> column pb*8 + m//128, partition m%128
    csmp = np.concatenate([s for _, s in cleanup]) if cleanup else np.empty(0, int)
    ccls = (
        np.concatenate([np.full(len(s), c) for c, s in cleanup])
        if cleanup
        else np.empty(0, int)
    )
    assert csmp.shape[0] == n_clean(pb), csmp.shape
    order[ncells * BCOL :] = csmp
    idx_stream[ncells:] = ccls
    return order, idx_stream


def make_in_maps(x, class_avg, target, pb=PB):
    x = np.ascontiguousarray(np.asarray(x, dtype=np.float32))
    ca = np.ascontiguousarray(np.asarray(class_avg, dtype=np.float32))
    tg = np.asarray(target).astype(np.int32)
    assert x.shape == (B, C) and ca.shape == (C, C) and tg.shape == (B,)
    # global class sort -> contiguous shards span ~C/8 classes each
    gsort = np.argsort(tg, kind="stable")
    maps = []
    for c in range(NCORES):
        rows = gsort[c * BLOC : (c + 1) * BLOC]
        xs, ts = x[rows], tg[rows]
        order, idx_stream = plan_shard(ts, pb)
        maps.append(
            {
                "x": np.ascontiguousarray(xs[order]),
                "ca": ca,
                "tgw": wrap_idx(idx_stream, pb),
            }
        )
    return maps


def pick_pb(target):
    """Largest pb all shards support (7 for any near-uniform targets)."""
    tg = np.asarray(target).astype(np.int32)
    gsort = np.argsort(tg, kind="stable")
    return min(
        max_pure_bands(tg[gsort[c * BLOC : (c + 1) * BLOC]]) for c in range(NCORES)
    )


def reduce_outputs(results):
    tot = 0.0
    for c in range(NCORES):
        tot += results[c]["out"].astype(np.float64).sum()
    return np.array(tot / B, dtype=np.float32)


def kernel(x, class_avg, target):
    pb = pick_pb(target)
    nc = get_program(pb)
    in_maps = make_in_maps(x, class_avg, target, pb)
    res = run_bass_kernel_spmd(nc, in_maps, list(range(NCORES)))
    return reduce_outputs(res.results)


# revision 38
# speedup vs baseline: 1.3960x; 1.0242x over previous
"""Class-conditional label-smoothing cross-entropy loss on 8 Trainium2 cores.

Reference math (C=1000 classes, B=65536 samples, smoothing s=0.1):
    A = softmax(class_avg, axis=-1)                         # [C, C]
    S[t, j] = s * (1 - A[t, j]) / (1 - A[t, t])  (j != t);  S[t, t] = 1 - s
    R[t]    = sum_j S[t, j]
    loss_i  = lse_i * R[t_i] - S[t_i] . x_i,   lse_i = log(sum_j exp(x_ij))
    out     = mean_i loss_i

The loss is a mean over samples, so sample order is free. The host sorts
the WHOLE batch by target class and cuts it into 8 contiguous shards, so
each core sees only ~126 distinct classes with ~65 samples each. Each
core's 128x64 slot grid (sample j*128+p -> partition p, column j) is then
packed into:
  - 7 "pure" bands of 8 columns: each (partition, band) cell holds 8
    samples of ONE class, so a single 896-row dma_gather serves all 56
    pure columns (class-run packing, host-planned).
  - 1 cleanup band (columns 56-63) for the leftover <8-sample class
    remnants and overflow runs, gathered per-sample (1024 rows, one
    dma_gather), class-sorted for HBM row-buffer locality.
This cuts per-core gather traffic from 8.39 MB (64 x 128 rows) to 1.9 MB
(896 + 1024 rows).

Each core:
  1. builds the smoothing table in its DRAM once:
     tab[t] = [S[t, :] as fp8e4 (1000 B) | R[t] as f32 bit-packed into 4
     fp8 slots | zero pad to 1024 B]  (fp8 quarters the per-sample gather
     traffic; since E[x]=0 the S quantization noise is mean-zero in the
     final scalar, and R stays exact f32 via the bit-pack)
  2. gathers use dma_gather (InstDMAGatherAnt, mlp library): ONE
     instruction gathers up to 1024 rows (vs one indirect_dma_start per
     column, whose ~1 us fixed SWDGE descriptor-emission cost serialized on
     the Pool engine and dominated the old schedule). dma_gather's native
     output layout dst[i%128, i//128, :] = tab[idx_i] matches the
     column-major slot layout exactly.
  3. per column: ACT exp with accumulate -> sumexp, one fused DVE multiply
     with accumulate -> dot; R (bit-packed f32) is copied once per band
     (free-dim broadcast) or per cleanup column.
  4. tail: lse = ln(sumexp), loss = R*lse - dot, one [128, 64] store.
Host sums the 8 partial grids in f64 and divides by B.
"""

import contextlib

import numpy as np

import concourse.bass as bass
import concourse.tile as tile
from concourse import bacc, library_config, mybir
from concourse.bass_utils import run_bass_kernel_spmd

B = 65536
C = 1000
NCORES = 8
BLOC = B // NCORES          # 8192 samples per core
P = 128
NT = BLOC // P              # 64 sample columns per core
TABW = 1024                 # table row: 1000 fp8 S + f32 R (4 slots) + pad
SM = 0.1
BCOL = 8                    # columns per band
PB = 7                      # pure bands (one gather per band), uniform case

_CACHE = {}
# Main-loop passes per hardware-loop iteration in the reps>1 timing build.
# KEEP AT 1: a 4-pass unrolled body measured 134.7 us/pass vs 111.9 for the
# 1-pass body (HW, congestion-immune looped-program diff) — the larger body
# appears to overflow the NX sequencers' instruction cache, so every
# For_i iteration re-streams it, stealing HBM bandwidth from the x loads.
UNROLL = 1


def n_passes(reps):
    """Main-loop passes executed by build_program(reps)."""
    return 1 if reps == 1 else reps * UNROLL


def n_clean(pb):
    return (NT - pb * BCOL) * P


def n_idx(pb):
    return pb * P + n_clean(pb)


def idx_w(pb):
    return n_idx(pb) // 16


def build_program(
    reps=1, abl=(), x_chunk=4, xs_bufs=5, gs_bufs=2, scr_bufs=6, pb=PB,
    unroll=UNROLL, merge_bands=True,
):
    # abl: timing-ablation switches ("gather" | "x" | "act" | "dve"), each
    # drops that component from the main loop (breaks numerics, timing only).
    # x_chunk: sample columns per x DMA (2 -> 1 MiB transfers).
    # reps>1 wraps the main loop in a tc.For_i HARDWARE loop (same data every
    # pass) for slope-timing in test.py: device time scales with reps while
    # compile time and dispatch overhead do not. The per-iteration all-engine
    # barrier is a pipeline drain the production reps=1 program does not
    # have, so the slope slightly OVER-estimates the production pass.
    f32 = mybir.dt.float32
    bf16 = mybir.dt.bfloat16
    i16 = mybir.dt.int16
    Alu = mybir.AluOpType
    Act = mybir.ActivationFunctionType
    tdt = mybir.dt.float8e4
    rslots = 4  # R occupies 4 fp8 slots right after the C S-entries

    IDXW = idx_w(pb)
    nc = bacc.Bacc("TRN2", target_bir_lowering=False, debug=False)
    x_ap = nc.dram_tensor("x", [BLOC, C], f32, kind="ExternalInput").ap()
    ca_ap = nc.dram_tensor("ca", [C, C], f32, kind="ExternalInput").ap()
    tg_ap = nc.dram_tensor("tgw", [P, IDXW], i16, kind="ExternalInput").ap()
    out_ap = nc.dram_tensor("out", [P, NT], f32, kind="ExternalOutput").ap()
    tab_ap = nc.dram_tensor("tab", [C, TABW], tdt).ap()

    with tile.TileContext(nc) as tc:
        nc.gpsimd.load_library(library_config.mlp)
        with (
            tc.tile_pool(name="tabp", bufs=2) as tabp,
            tc.tile_pool(name="small", bufs=2) as small,
            tc.tile_pool(name="xs", bufs=xs_bufs) as xs,
            tc.tile_pool(name="gbp", bufs=gs_bufs) as gbp,
            tc.tile_pool(name="gcp", bufs=2) as gcp,
            tc.tile_pool(name="scr", bufs=scr_bufs) as scr,
            tc.tile_pool(name="cols", bufs=1) as cols,
        ):
            # wrapped+replicated target indices for dma_gather
            idx = cols.tile([P, IDXW], i16)
            nc.sync.dma_start(idx[:], tg_ap[:, :])

            # ---- smoothing table -------------------------------------------
            for k in range((C + P - 1) // P):
                r0 = k * P
                pr = min(r0 + P, C) - r0
                cat = tabp.tile([P, C], f32, tag="cat")
                nc.sync.dma_start(cat[:pr], ca_ap[r0 : r0 + pr, :])
                e = tabp.tile([P, C], f32, tag="e")
                sume = small.tile([P, 1], f32, tag="sume")
                nc.scalar.activation(e[:pr], cat[:pr], Act.Exp, accum_out=sume[:pr])
                # diagonal e[t, t] via affine mask + row reduce
                msk = tabp.tile([P, C], f32, tag="msk")
                nc.gpsimd.affine_select(
                    out=msk[:pr], in_=e[:pr], compare_op=Alu.is_equal, fill=0.0,
                    base=-r0, channel_multiplier=-1, pattern=[[1, C]],
                )
                ett = small.tile([P, 1], f32, tag="ett")
                nc.vector.tensor_reduce(
                    out=ett[:pr], in_=msk[:pr], axis=mybir.AxisListType.X, op=Alu.add
                )
                den = small.tile([P, 1], f32, tag="den")
                nc.vector.tensor_tensor(
                    out=den[:pr], in0=sume[:pr], in1=ett[:pr], op=Alu.subtract
                )
                rec = small.tile([P, 1], f32, tag="rec")
                nc.vector.reciprocal(rec[:pr], den[:pr])
                negw = small.tile([P, 1], f32, tag="negw")
                nc.vector.tensor_scalar_mul(negw[:pr], rec[:pr], -SM)
                # S_pre[t, j] = (e - sume) * (-s / den); its diagonal equals s,
                # and sum_j S_pre = R - (1 - 2s)
                spre = tabp.tile([P, C], f32, tag="spre")
                rpre = small.tile([P, 1], f32, tag="rpre")
                nc.vector.scalar_tensor_tensor(
                    out=spre[:pr], in0=e[:pr], scalar=sume[:pr],
                    in1=negw[:pr].to_broadcast([pr, C]),
                    op0=Alu.subtract, op1=Alu.mult, accum_out=rpre[:pr],
                )
                sb = tabp.tile([P, TABW], tdt, tag="sb")
                nc.gpsimd.affine_select(
                    out=sb[:pr, 0:C], in_=spre[:pr], compare_op=Alu.not_equal,
                    fill=1.0 - SM, base=-r0, channel_multiplier=-1, pattern=[[1, C]],
                )
                # R as raw f32 bits in the 4 fp8 slots after the S entries
                rt = small.tile([P, 1], f32, tag="rt")
                nc.vector.tensor_scalar_add(rt[:pr], rpre[:pr], 1.0 - 2 * SM)
                rv = sb[:pr, C : C + rslots].bitcast(f32)
                nc.vector.tensor_copy(out=rv[:, 0:1], in_=rt[:pr])
                nc.vector.memset(sb[:pr, C + rslots : TABW], 0.0)
                nc.sync.dma_start(tab_ap[r0 : r0 + pr, :], sb[:pr])

            # ---- main loop -------------------------------------------------
            # x viewed column-major: sample j*128 + p -> partition p, col j,
            # so an x_chunk load is one DMA over a contiguous DRAM range
            x_r = x_ap.rearrange("(c p) d -> p c d", p=P)
            se_cols = cols.tile([P, NT], f32)
            dot_cols = cols.tile([P, NT], f32)
            r_cols = cols.tile([P, NT], f32)
            if abl:
                nc.vector.memset(se_cols[:], 1.0)
                nc.vector.memset(dot_cols[:], 1.0)
                nc.vector.memset(r_cols[:], 1.0)
            xt0 = gt0 = None
            if "x" in abl:
                xt0 = cols.tile([P, C], f32)
                nc.sync.dma_start(xt0[:], x_r[:, 0, :])
            if "gather" in abl:
                gt0 = cols.tile([P, 1, TABW], tdt)
                nc.vector.memset(gt0[:, 0, :], 0.25)
            xbig = gband = gclean = None
            loop = tc.For_i(0, reps) if reps > 1 else contextlib.nullcontext()
            with loop:
              for j in range(NT * (unroll if reps > 1 else 1)):
                j = j % NT
                if "x" in abl:
                    xt = xt0
                else:
                    if j % x_chunk == 0:
                        xbig = xs.tile([P, x_chunk, C], f32)
                        nc.sync.dma_start(xbig[:], x_r[:, j : j + x_chunk, :])
                    xt = xbig[:, j % x_chunk, :]
                in_pure = j < pb * BCOL
                new_band = in_pure and j % BCOL == 0
                if "gather" in abl:
                    gt = gt0[:, 0, :]
                elif in_pure:
                    b = j // BCOL
                    if merge_bands:
                        if j == 0:
                            gband = gbp.tile([P, pb, TABW], tdt, tag="gband")
                            nc.gpsimd.dma_gather(
                                gband[:], tab_ap[:],
                                idx[:, 0 : pb * (P // 16)],
                                pb * P, pb * P, TABW,
                            )
                        gt = gband[:, b, :]
                    else:
                        if new_band:
                            # one row per (partition, band) cell, reused 8 cols
                            gband = gbp.tile([P, 1, TABW], tdt, tag="gband")
                            nc.gpsimd.dma_gather(
                                gband[:], tab_ap[:],
                                idx[:, b * (P // 16) : (b + 1) * (P // 16)],
                                P, P, TABW,
                            )
                        gt = gband[:, 0, :]
                else:
                    jc = j - pb * BCOL
                    if jc % BCOL == 0:
                        # per-sample gather, one 8-column chunk at a time
                        gclean = gcp.tile([P, BCOL, TABW], tdt, tag="gc")
                        c0 = pb * (P // 16) + jc * (P // 16)
                        nc.gpsimd.dma_gather(
                            gclean[:], tab_ap[:],
                            idx[:, c0 : c0 + BCOL * (P // 16)],
                            BCOL * P, BCOL * P, TABW,
                        )
                    gt = gclean[:, jc % BCOL, :]
                if "act" not in abl:
                    es = scr.tile([P, C], bf16, tag="es")
                    nc.scalar.activation(
                        es[:], xt[:], Act.Exp, accum_out=se_cols[:, j : j + 1]
                    )
                if "dve" not in abl:
                    ps = scr.tile([P, C], f32, tag="ps")
                    nc.vector.scalar_tensor_tensor(
                        out=ps[:], in0=xt[:], scalar=1.0, in1=gt[:, 0:C],
                        op0=Alu.mult, op1=Alu.mult, accum_out=dot_cols[:, j : j + 1],
                    )
                    grv = gt[:, C : C + rslots].bitcast(f32)
                    if new_band:
                        nc.vector.tensor_copy(
                            out=r_cols[:, j : j + BCOL],
                            in_=grv[:, 0:1].to_broadcast([P, BCOL]),
                        )
                    elif not in_pure:
                        nc.vector.tensor_copy(
                            out=r_cols[:, j : j + 1], in_=grv[:, 0:1]
                        )

            # ---- tail ------------------------------------------------------
            lse = cols.tile([P, NT], f32)
            nc.scalar.activation(lse[:], se_cols[:], Act.Ln)
            t1 = cols.tile([P, NT], f32)
            nc.vector.tensor_mul(t1[:], r_cols[:], lse[:])
            loss = cols.tile([P, NT], f32)
            nc.vector.tensor_tensor(
                out=loss[:], in0=t1[:], in1=dot_cols[:], op=Alu.subtract
            )
            nc.sync.dma_start(out_ap[:], loss[:])

    nc.compile()
    nc.finalize()
    return nc


def get_program(pb=PB):
    key = ("nc", pb)
    if key not in _CACHE:
        _CACHE[key] = build_program(pb=pb)
    return _CACHE[key]


def wrap_idx(stream, pb=PB):
    """[NIDX] int -> [128, NIDX/16] int16 wrapped (pos i -> [i%16, i//16])
    and replicated across the 8 partition groups of 16 (one per Q7 core)."""
    w = stream.astype(np.int16).reshape(idx_w(pb), 16).T  # [16, IDXW]
    return np.ascontiguousarray(np.tile(w, (8, 1)))  # [128, IDXW]


def max_pure_bands(ts):
    """Max pure bands a shard supports: full 8-run count // 128, capped."""
    cnt = np.bincount(ts, minlength=C)
    return int(min(PB, (cnt // BCOL).sum() // P))


def plan_shard(ts, pb=PB):
    """Pack one class-sorted shard into the slot grid.

    Returns (order, idx_stream): order[slot_linear] = shard sample row for
    device slot (p, j) with slot_linear = j*128 + p; idx_stream[n_idx(pb)] =
    the gather-index stream (pb*128 band cells then cleanup samples).
    """
    n = ts.shape[0]
    assert n == BLOC
    srt = np.argsort(ts, kind="stable")
    tss = ts[srt]
    # class group boundaries in the sorted shard
    bounds = np.flatnonzero(np.r_[True, tss[1:] != tss[:-1], True])
    runs = []          # full 8-sample runs: (class, 8 sample rows)
    cleanup = []       # leftover sample rows (class-sorted order)
    ncells = pb * P
    for g0, g1 in zip(bounds[:-1], bounds[1:]):
        cls = int(tss[g0])
        k = g0
        while k + BCOL <= g1 and len(runs) < ncells:
            runs.append((cls, srt[k : k + BCOL]))
            k += BCOL
        if k < g1:
            cleanup.append((cls, srt[k:g1]))
    assert len(runs) == ncells, (
        f"class-run packing infeasible: {len(runs)} full runs < {ncells}"
    )
    order = np.empty(n, dtype=np.int64)
    idx_stream = np.empty(n_idx(pb), dtype=np.int64)
    if ncells:
        # run r -> band b = r // 128, partition p = r % 128, cols b*8..b*8+7
        rcls = np.array([c for c, _ in runs])
        rsmp = np.stack([s for _, s in runs])          # [ncells, 8]
        r = np.arange(ncells)
        lin = ((r[:, None] // P) * BCOL + np.arange(BCOL)[None, :]) * P + (
            r[:, None] % P
        )
        order[lin.ravel()] = rsmp.ravel()
        idx_stream[:ncells] = rcls
    # cleanup sample m -> column pb*8 + m//128, partition m%128
    csmp = np.concatenate([s for _, s in cleanup]) if cleanup else np.empty(0, int)
    ccls = (
        np.concatenate([np.full(len(s), c) for c, s in cleanup])
        if cleanup
        else np.empty(0, int)
    )
    assert csmp.shape[0] == n_clean(pb), csmp.shape
    order[ncells * BCOL :] = csmp
    idx_stream[ncells:] = ccls
    return order, idx_stream


def make_in_maps(x, class_avg, target, pb=PB):
    x = np.ascontiguousarray(np.asarray(x, dtype=np.float32))
    ca = np.ascontiguousarray(np.asarray(class_avg, dtype=np.float32))
    tg = np.asarray(target).astype(np.int32)
    assert x.shape == (B, C) and ca.shape == (C, C) and tg.shape == (B,)
    # global class sort -> contiguous shards span ~C/8 classes each
    gsort = np.argsort(tg, kind="stable")
    maps = []
    for c in range(NCORES):
        rows = gsort[c * BLOC : (c + 1) * BLOC]
        xs, ts = x[rows], tg[rows]
        order, idx_stream = plan_shard(ts, pb)
        maps.append(
            {
                "x": np.ascontiguousarray(xs[order]),
                "ca": ca,
                "tgw": wrap_idx(idx_stream, pb),
            }
        )
    return maps


def pick_pb(target):
    """Largest pb all shards support (7 for any near-uniform targets)."""
    tg = np.asarray(target).astype(np.int32)
    gsort = np.argsort(tg, kind="stable")
    return min(
        max_pure_bands(tg[gsort[c * BLOC : (c + 1) * BLOC]]) for c in range(NCORES)
    )


def reduce_outputs(results):
    tot = 0.0
    for c in range(NCORES):
        tot += results[c]["out"].astype(np.float64).sum()
    return np.array(tot / B, dtype=np.float32)


def kernel(x, class_avg, target):
    pb = pick_pb(target)
    nc = get_program(pb)
    in_maps = make_in_maps(x, class_avg, target, pb)
    res = run_bass_kernel_spmd(nc, in_maps, list(range(NCORES)))
    return reduce_outputs(res.results)


# revision 41
# speedup vs baseline: 1.4410x; 1.0322x over previous
"""Class-conditional label-smoothing cross-entropy loss on 8 Trainium2 cores.

Reference math (C=1000 classes, B=65536 samples, smoothing s=0.1):
    A = softmax(class_avg, axis=-1)                         # [C, C]
    S[t, j] = s * (1 - A[t, j]) / (1 - A[t, t])  (j != t);  S[t, t] = 1 - s
    R[t]    = sum_j S[t, j]
    loss_i  = lse_i * R[t_i] - S[t_i] . x_i,   lse_i = log(sum_j exp(x_ij))
    out     = mean_i loss_i

The loss is a mean over samples, so sample order is free. The host sorts
the WHOLE batch by target class and cuts it into 8 contiguous shards, so
each core sees only ~126 distinct classes with ~65 samples each. Each
core's 128x64 slot grid (sample j*128+p -> partition p, column j) is then
packed into:
  - 7 "pure" bands of 8 columns: each (partition, band) cell holds 8
    samples of ONE class, so a single 896-row dma_gather serves all 56
    pure columns (class-run packing, host-planned).
  - 1 cleanup band (columns 56-63) for the leftover <8-sample class
    remnants and overflow runs, gathered per-sample (1024 rows, one
    dma_gather), class-sorted for HBM row-buffer locality.
This cuts per-core gather traffic from 8.39 MB (64 x 128 rows) to 1.9 MB
(896 + 1024 rows).

Each core:
  1. builds the smoothing table in its DRAM once:
     tab[t] = [S[t, :] as fp8e4 (1000 B) | R[t] as f32 bit-packed into 4
     fp8 slots | zero pad to 1024 B]  (fp8 quarters the per-sample gather
     traffic; since E[x]=0 the S quantization noise is mean-zero in the
     final scalar, and R stays exact f32 via the bit-pack)
  2. gathers use dma_gather (InstDMAGatherAnt, mlp library): ONE
     instruction gathers up to 1024 rows (vs one indirect_dma_start per
     column, whose ~1 us fixed SWDGE descriptor-emission cost serialized on
     the Pool engine and dominated the old schedule). dma_gather's native
     output layout dst[i%128, i//128, :] = tab[idx_i] matches the
     column-major slot layout exactly.
  3. per column: ACT exp with accumulate -> sumexp, one fused DVE multiply
     with accumulate -> dot; R (bit-packed f32) is copied once per band
     (free-dim broadcast) or per cleanup column.
  4. tail: lse = ln(sumexp), loss = R*lse - dot, one [128, 64] store.
Host sums the 8 partial grids in f64 and divides by B.
"""

import contextlib

import numpy as np

import concourse.bass as bass
import concourse.tile as tile
from concourse import bacc, library_config, mybir
from concourse.bass_utils import run_bass_kernel_spmd

B = 65536
C = 1000
NCORES = 8
BLOC = B // NCORES          # 8192 samples per core
P = 128
NT = BLOC // P              # 64 sample columns per core
TABW = 1024                 # table row: 1000 fp8 S + f32 R (4 slots) + pad
SM = 0.1
BCOL = 8                    # columns per band
PB = 7                      # pure bands (one gather per band), uniform case

_CACHE = {}
# Main-loop passes per hardware-loop iteration in the reps>1 timing build.
# KEEP AT 1: a 4-pass unrolled body measured 134.7 us/pass vs 111.9 for the
# 1-pass body (HW, congestion-immune looped-program diff) — the larger body
# appears to overflow the NX sequencers' instruction cache, so every
# For_i iteration re-streams it, stealing HBM bandwidth from the x loads.
UNROLL = 1


def n_passes(reps):
    """Main-loop passes executed by build_program(reps)."""
    return 1 if reps == 1 else reps * UNROLL


def n_clean(pb):
    return (NT - pb * BCOL) * P


def n_idx(pb):
    return pb * P + n_clean(pb)


def idx_w(pb):
    return n_idx(pb) // 16


def build_program(
    reps=1, abl=(), x_chunk=4, xs_bufs=5, gs_bufs=2, scr_bufs=6, pb=PB,
    unroll=UNROLL, merge_bands=True, taper=False,
):
    # abl: timing-ablation switches ("gather" | "x" | "act" | "dve"), each
    # drops that component from the main loop (breaks numerics, timing only).
    # x_chunk: sample columns per x DMA (2 -> 1 MiB transfers).
    # reps>1 wraps the main loop in a tc.For_i HARDWARE loop (same data every
    # pass) for slope-timing in test.py: device time scales with reps while
    # compile time and dispatch overhead do not. The per-iteration all-engine
    # barrier is a pipeline drain the production reps=1 program does not
    # have, so the slope slightly OVER-estimates the production pass.
    f32 = mybir.dt.float32
    bf16 = mybir.dt.bfloat16
    i16 = mybir.dt.int16
    Alu = mybir.AluOpType
    Act = mybir.ActivationFunctionType
    tdt = mybir.dt.float8e4
    rslots = 4  # R occupies 4 fp8 slots right after the C S-entries

    IDXW = idx_w(pb)
    nc = bacc.Bacc("TRN2", target_bir_lowering=False, debug=False)
    x_ap = nc.dram_tensor("x", [BLOC, C], f32, kind="ExternalInput").ap()
    ca_ap = nc.dram_tensor("ca", [C, C], f32, kind="ExternalInput").ap()
    tg_ap = nc.dram_tensor("tgw", [P, IDXW], i16, kind="ExternalInput").ap()
    out_ap = nc.dram_tensor("out", [P, NT], f32, kind="ExternalOutput").ap()
    tab_ap = nc.dram_tensor("tab", [C, TABW], tdt).ap()

    with tile.TileContext(nc) as tc:
        nc.gpsimd.load_library(library_config.mlp)
        with (
            tc.tile_pool(name="tabp", bufs=2) as tabp,
            tc.tile_pool(name="small", bufs=2) as small,
            tc.tile_pool(name="xs", bufs=xs_bufs) as xs,
            tc.tile_pool(name="gbp", bufs=gs_bufs) as gbp,
            tc.tile_pool(name="gcp", bufs=2) as gcp,
            tc.tile_pool(name="scr", bufs=scr_bufs) as scr,
            tc.tile_pool(name="cols", bufs=1) as cols,
        ):
            # wrapped+replicated target indices for dma_gather
            idx = cols.tile([P, IDXW], i16)
            nc.sync.dma_start(idx[:], tg_ap[:, :])

            # ---- smoothing table -------------------------------------------
            for k in range((C + P - 1) // P):
                r0 = k * P
                pr = min(r0 + P, C) - r0
                cat = tabp.tile([P, C], f32, tag="cat")
                nc.sync.dma_start(cat[:pr], ca_ap[r0 : r0 + pr, :])
                e = tabp.tile([P, C], f32, tag="e")
                sume = small.tile([P, 1], f32, tag="sume")
                nc.scalar.activation(e[:pr], cat[:pr], Act.Exp, accum_out=sume[:pr])
                # diagonal e[t, t] via affine mask + row reduce
                msk = tabp.tile([P, C], f32, tag="msk")
                nc.gpsimd.affine_select(
                    out=msk[:pr], in_=e[:pr], compare_op=Alu.is_equal, fill=0.0,
                    base=-r0, channel_multiplier=-1, pattern=[[1, C]],
                )
                ett = small.tile([P, 1], f32, tag="ett")
                nc.vector.tensor_reduce(
                    out=ett[:pr], in_=msk[:pr], axis=mybir.AxisListType.X, op=Alu.add
                )
                den = small.tile([P, 1], f32, tag="den")
                nc.vector.tensor_tensor(
                    out=den[:pr], in0=sume[:pr], in1=ett[:pr], op=Alu.subtract
                )
                rec = small.tile([P, 1], f32, tag="rec")
                nc.vector.reciprocal(rec[:pr], den[:pr])
                negw = small.tile([P, 1], f32, tag="negw")
                nc.vector.tensor_scalar_mul(negw[:pr], rec[:pr], -SM)
                # S_pre[t, j] = (e - sume) * (-s / den); its diagonal equals s,
                # and sum_j S_pre = R - (1 - 2s)
                spre = tabp.tile([P, C], f32, tag="spre")
                rpre = small.tile([P, 1], f32, tag="rpre")
                nc.vector.scalar_tensor_tensor(
                    out=spre[:pr], in0=e[:pr], scalar=sume[:pr],
                    in1=negw[:pr].to_broadcast([pr, C]),
                    op0=Alu.subtract, op1=Alu.mult, accum_out=rpre[:pr],
                )
                sb = tabp.tile([P, TABW], tdt, tag="sb")
                nc.gpsimd.affine_select(
                    out=sb[:pr, 0:C], in_=spre[:pr], compare_op=Alu.not_equal,
                    fill=1.0 - SM, base=-r0, channel_multiplier=-1, pattern=[[1, C]],
                )
                # R as raw f32 bits in the 4 fp8 slots after the S entries
                rt = small.tile([P, 1], f32, tag="rt")
                nc.vector.tensor_scalar_add(rt[:pr], rpre[:pr], 1.0 - 2 * SM)
                rv = sb[:pr, C : C + rslots].bitcast(f32)
                nc.vector.tensor_copy(out=rv[:, 0:1], in_=rt[:pr])
                nc.vector.memset(sb[:pr, C + rslots : TABW], 0.0)
                nc.sync.dma_start(tab_ap[r0 : r0 + pr, :], sb[:pr])

            # ---- main loop -------------------------------------------------
            # x viewed column-major: sample j*128 + p -> partition p, col j,
            # so an x_chunk load is one DMA over a contiguous DRAM range
            x_r = x_ap.rearrange("(c p) d -> p c d", p=P)
            se_cols = cols.tile([P, NT], f32)
            dot_cols = cols.tile([P, NT], f32)
            r_cols = cols.tile([P, NT], f32)
            if abl:
                nc.vector.memset(se_cols[:], 1.0)
                nc.vector.memset(dot_cols[:], 1.0)
                nc.vector.memset(r_cols[:], 1.0)
            xt0 = gt0 = None
            if "x" in abl:
                xt0 = cols.tile([P, C], f32)
                nc.sync.dma_start(xt0[:], x_r[:, 0, :])
            if "gather" in abl:
                gt0 = cols.tile([P, 1, TABW], tdt)
                nc.vector.memset(gt0[:, 0, :], 0.25)
            # x chunk start -> size; taper shrinks the trailing chunks so the
            # end-of-pass drain (last chunk's compute + tail with DMA idle)
            # covers 1 column instead of x_chunk
            sizes = [x_chunk] * (NT // x_chunk)
            if taper and x_chunk == 4:
                sizes = sizes[:-2] + [2, 2, 2, 1, 1]
            chunk_at = {}
            j0 = 0
            for sz in sizes:
                chunk_at[j0] = sz
                j0 += sz
            assert j0 == NT
            xbig = gband = gclean = None
            x0 = 0
            loop = tc.For_i(0, reps) if reps > 1 else contextlib.nullcontext()
            with loop:
              for j in range(NT * (unroll if reps > 1 else 1)):
                j = j % NT
                if "x" in abl:
                    xt = xt0
                else:
                    if j in chunk_at:
                        sz = chunk_at[j]
                        # uniform max-size tile so the pool ring stays regular
                        xbig = xs.tile([P, x_chunk, C], f32)
                        nc.sync.dma_start(xbig[:, 0:sz, :], x_r[:, j : j + sz, :])
                        x0 = j
                    xt = xbig[:, j - x0, :]
                in_pure = j < pb * BCOL
                new_band = in_pure and j % BCOL == 0
                if "gather" in abl:
                    gt = gt0[:, 0, :]
                elif in_pure:
                    b = j // BCOL
                    if merge_bands:
                        if j == 0:
                            gband = gbp.tile([P, pb, TABW], tdt, tag="gband")
                            nc.gpsimd.dma_gather(
                                gband[:], tab_ap[:],
                                idx[:, 0 : pb * (P // 16)],
                                pb * P, pb * P, TABW,
                            )
                        gt = gband[:, b, :]
                    else:
                        if new_band:
                            # one row per (partition, band) cell, reused 8 cols
                            gband = gbp.tile([P, 1, TABW], tdt, tag="gband")
                            nc.gpsimd.dma_gather(
                                gband[:], tab_ap[:],
                                idx[:, b * (P // 16) : (b + 1) * (P // 16)],
                                P, P, TABW,
                            )
                        gt = gband[:, 0, :]
                else:
                    jc = j - pb * BCOL
                    if jc % BCOL == 0:
                        # per-sample gather, one 8-column chunk at a time
                        gclean = gcp.tile([P, BCOL, TABW], tdt, tag="gc")
                        c0 = pb * (P // 16) + jc * (P // 16)
                        nc.gpsimd.dma_gather(
                            gclean[:], tab_ap[:],
                            idx[:, c0 : c0 + BCOL * (P // 16)],
                            BCOL * P, BCOL * P, TABW,
                        )
                    gt = gclean[:, jc % BCOL, :]
                if "act" not in abl:
                    es = scr.tile([P, C], bf16, tag="es")
                    nc.scalar.activation(
                        es[:], xt[:], Act.Exp, accum_out=se_cols[:, j : j + 1]
                    )
                if "dve" not in abl:
                    ps = scr.tile([P, C], f32, tag="ps")
                    nc.vector.scalar_tensor_tensor(
                        out=ps[:], in0=xt[:], scalar=1.0, in1=gt[:, 0:C],
                        op0=Alu.mult, op1=Alu.mult, accum_out=dot_cols[:, j : j + 1],
                    )
                    grv = gt[:, C : C + rslots].bitcast(f32)
                    if new_band:
                        nc.vector.tensor_copy(
                            out=r_cols[:, j : j + BCOL],
                            in_=grv[:, 0:1].to_broadcast([P, BCOL]),
                        )
                    elif not in_pure:
                        nc.vector.tensor_copy(
                            out=r_cols[:, j : j + 1], in_=grv[:, 0:1]
                        )

            # ---- tail ------------------------------------------------------
            lse = cols.tile([P, NT], f32)
            nc.scalar.activation(lse[:], se_cols[:], Act.Ln)
            t1 = cols.tile([P, NT], f32)
            nc.vector.tensor_mul(t1[:], r_cols[:], lse[:])
            loss = cols.tile([P, NT], f32)
            nc.vector.tensor_tensor(
                out=loss[:], in0=t1[:], in1=dot_cols[:], op=Alu.subtract
            )
            nc.sync.dma_start(out_ap[:], loss[:])

    nc.compile()
    nc.finalize()
    return nc


def get_program(pb=PB):
    key = ("nc", pb)
    if key not in _CACHE:
        _CACHE[key] = build_program(pb=pb)
    return _CACHE[key]


def wrap_idx(stream, pb=PB):
    """[NIDX] int -> [128, NIDX/16] int16 wrapped (pos i -> [i%16, i//16])
    and replicated across the 8 partition groups of 16 (one per Q7 core)."""
    w = stream.astype(np.int16).reshape(idx_w(pb), 16).T  # [16, IDXW]
    return np.ascontiguousarray(np.tile(w, (8, 1)))  # [128, IDXW]


def max_pure_bands(ts):
    """Max pure bands a shard supports: full 8-run count // 128, capped."""
    cnt = np.bincount(ts, minlength=C)
    return int(min(PB, (cnt // BCOL).sum() // P))


def plan_shard(ts, pb=PB):
    """Pack one class-sorted shard into the slot grid.

    Returns (order, idx_stream): order[slot_linear] = shard sample row for
    device slot (p, j) with slot_linear = j*128 + p; idx_stream[n_idx(pb)] =
    the gather-index stream (pb*128 band cells then cleanup samples).
    """
    n = ts.shape[0]
    assert n == BLOC
    srt = np.argsort(ts, kind="stable")
    tss = ts[srt]
    # class group boundaries in the sorted shard
    bounds = np.flatnonzero(np.r_[True, tss[1:] != tss[:-1], True])
    runs = []          # full 8-sample runs: (class, 8 sample rows)
    cleanup = []       # leftover sample rows (class-sorted order)
    ncells = pb * P
    for g0, g1 in zip(bounds[:-1], bounds[1:]):
        cls = int(tss[g0])
        k = g0
        while k + BCOL <= g1 and len(runs) < ncells:
            runs.append((cls, srt[k : k + BCOL]))
            k += BCOL
        if k < g1:
            cleanup.append((cls, srt[k:g1]))
    assert len(runs) == ncells, (
        f"class-run packing infeasible: {len(runs)} full runs < {ncells}"
    )
    order = np.empty(n, dtype=np.int64)
    idx_stream = np.empty(n_idx(pb), dtype=np.int64)
    if ncells:
        # run r -> band b = r // 128, partition p = r % 128, cols b*8..b*8+7
        rcls = np.array([c for c, _ in runs])
        rsmp = np.stack([s for _, s in runs])          # [ncells, 8]
        r = np.arange(ncells)
        lin = ((r[:, None] // P) * BCOL + np.arange(BCOL)[None, :]) * P + (
            r[:, None] % P
        )
        order[lin.ravel()] = rsmp.ravel()
        idx_stream[:ncells] = rcls
    # cleanup sample m -> column pb*8 + m//128, partition m%128
    csmp = np.concatenate([s for _, s in cleanup]) if cleanup else np.empty(0, int)
    ccls = (
        np.concatenate([np.full(len(s), c) for c, s in cleanup])
        if cleanup
        else np.empty(0, int)
    )
    assert csmp.shape[0] == n_clean(pb), csmp.shape
    order[ncells * BCOL :] = csmp
    idx_stream[ncells:] = ccls
    return order, idx_stream


def make_in_maps(x, class_avg, target, pb=PB):
    x = np.ascontiguousarray(np.asarray(x, dtype=np.float32))
    ca = np.ascontiguousarray(np.asarray(class_avg, dtype=np.float32))
    tg = np.asarray(target).astype(np.int32)
    assert x.shape == (B, C) and ca.shape == (C, C) and tg.shape == (B,)
    # global class sort -> contiguous shards span ~C/8 classes each
    gsort = np.argsort(tg, kind="stable")
    maps = []
    for c in range(NCORES):
        rows = gsort[c * BLOC : (c + 1) * BLOC]
        xs, ts = x[rows], tg[rows]
        order, idx_stream = plan_shard(ts, pb)
        maps.append(
            {
                "x": np.ascontiguousarray(xs[order]),
                "ca": ca,
                "tgw": wrap_idx(idx_stream, pb),
            }
        )
    return maps


def pick_pb(target):
    """Largest pb all shards support (7 for any near-uniform targets)."""
    tg = np.asarray(target).astype(np.int32)
    gsort = np.argsort(tg, kind="stable")
    return min(
        max_pure_bands(tg[gsort[c * BLOC : (c + 1) * BLOC]]) for c in range(NCORES)
    )


def reduce_outputs(results):
    tot = 0.0
    for c in range(NCORES):
        tot += results[c]["out"].astype(np.float64).sum()
    return np.array(tot / B, dtype=np.float32)


def kernel(x, class_avg, target):
    pb = pick_pb(target)
    nc = get_program(pb)
    in_maps = make_in_maps(x, class_avg, target, pb)
    res = run_bass_kernel_spmd(nc, in_maps, list(range(NCORES)))
    return reduce_outputs(res.results)


# revision 45
# speedup vs baseline: 1.6817x; 1.1671x over previous
"""Class-conditional label-smoothing cross-entropy loss on 8 Trainium2 cores.

Reference math (C=1000 classes, B=65536 samples, smoothing s=0.1):
    A = softmax(class_avg, axis=-1)                         # [C, C]
    S[t, j] = s * (1 - A[t, j]) / (1 - A[t, t])  (j != t);  S[t, t] = 1 - s
    R[t]    = sum_j S[t, j]
    loss_i  = lse_i * R[t_i] - S[t_i] . x_i,   lse_i = log(sum_j exp(x_ij))
    out     = mean_i loss_i

The loss is a mean over samples, so sample order is free. The host sorts
the WHOLE batch by target class and cuts it into 8 contiguous shards, so
each core sees only ~126 distinct classes with ~65 samples each. Each
core's 128x64 slot grid (sample j*128+p -> partition p, column j) is then
packed into:
  - 7 "pure" bands of 8 columns: each (partition, band) cell holds 8
    samples of ONE class, so a single 896-row dma_gather serves all 56
    pure columns (class-run packing, host-planned).
  - 1 cleanup band (columns 56-63) for the leftover <8-sample class
    remnants and overflow runs, gathered per-sample (1024 rows, one
    dma_gather), class-sorted for HBM row-buffer locality.
This cuts per-core gather traffic from 8.39 MB (64 x 128 rows) to 1.9 MB
(896 + 1024 rows).

Each core:
  1. builds the smoothing table in its DRAM once:
     tab[t] = [S[t, :] as fp8e4 (1000 B) | R[t] as f32 bit-packed into 4
     fp8 slots | zero pad to 1024 B]  (fp8 quarters the per-sample gather
     traffic; since E[x]=0 the S quantization noise is mean-zero in the
     final scalar, and R stays exact f32 via the bit-pack)
  2. gathers use dma_gather (InstDMAGatherAnt, mlp library): ONE
     instruction gathers up to 1024 rows (vs one indirect_dma_start per
     column, whose ~1 us fixed SWDGE descriptor-emission cost serialized on
     the Pool engine and dominated the old schedule). dma_gather's native
     output layout dst[i%128, i//128, :] = tab[idx_i] matches the
     column-major slot layout exactly.
  3. per column: ACT exp with accumulate -> sumexp, one fused DVE multiply
     with accumulate -> dot; R (bit-packed f32) is copied once per band
     (free-dim broadcast) or per cleanup column.
  4. tail: lse = ln(sumexp), loss = R*lse - dot, one [128, 64] store.
Host sums the 8 partial grids in f64 and divides by B.
"""

import contextlib

import numpy as np

import concourse.bass as bass
import concourse.tile as tile
from concourse import bacc, library_config, mybir
from concourse.bass_utils import run_bass_kernel_spmd

B = 65536
C = 1000
NCORES = 8
BLOC = B // NCORES          # 8192 samples per core
P = 128
NT = BLOC // P              # 64 sample columns per core
TABW = 1024                 # table row: 1000 fp8 S + f32 R (4 slots) + pad
SM = 0.1
BCOL = 8                    # columns per band
PB = 7                      # pure bands (one gather per band), uniform case

_CACHE = {}
# Main-loop passes per hardware-loop iteration in the reps>1 timing build.
# KEEP AT 1: a 4-pass unrolled body measured 134.7 us/pass vs 111.9 for the
# 1-pass body (HW, congestion-immune looped-program diff) — the larger body
# appears to overflow the NX sequencers' instruction cache, so every
# For_i iteration re-streams it, stealing HBM bandwidth from the x loads.
UNROLL = 1


def n_passes(reps):
    """Main-loop passes executed by build_program(reps)."""
    return 1 if reps == 1 else reps * UNROLL


def n_clean(pb):
    return (NT - pb * BCOL) * P


def n_idx(pb):
    return pb * P + n_clean(pb)


def idx_w(pb):
    return n_idx(pb) // 16


def build_program(
    reps=1, abl=(), x_chunk=4, xs_bufs=5, gs_bufs=2, scr_bufs=6, pb=PB,
    unroll=UNROLL, merge_bands=True, taper=False,
):
    # abl: timing-ablation switches ("gather" | "x" | "act" | "dve"), each
    # drops that component from the main loop (breaks numerics, timing only).
    # x_chunk: sample columns per x DMA (2 -> 1 MiB transfers).
    # reps>1 wraps the main loop in a tc.For_i HARDWARE loop (same data every
    # pass) for slope-timing in test.py: device time scales with reps while
    # compile time and dispatch overhead do not. The per-iteration all-engine
    # barrier is a pipeline drain the production reps=1 program does not
    # have, so the slope slightly OVER-estimates the production pass.
    f32 = mybir.dt.float32
    bf16 = mybir.dt.bfloat16
    i16 = mybir.dt.int16
    Alu = mybir.AluOpType
    Act = mybir.ActivationFunctionType
    tdt = mybir.dt.float8e4
    rslots = 4  # R occupies 4 fp8 slots right after the C S-entries

    IDXW = idx_w(pb)
    nc = bacc.Bacc("TRN2", target_bir_lowering=False, debug=False)
    # x is pre-cast to bf16 on the HOST (make_in_maps): halves the dominant
    # HBM stream (32.77 -> 16.4 MB/core). E[x]=0 and B=65536, so the
    # mean-zero bf16 rounding noise vanishes in the final scalar (same
    # argument as the fp8 S table).
    x_ap = nc.dram_tensor("x", [BLOC, C], bf16, kind="ExternalInput").ap()
    ca_ap = nc.dram_tensor("ca", [C, C], f32, kind="ExternalInput").ap()
    tg_ap = nc.dram_tensor("tgw", [P, IDXW], i16, kind="ExternalInput").ap()
    out_ap = nc.dram_tensor("out", [P, NT], f32, kind="ExternalOutput").ap()
    tab_ap = nc.dram_tensor("tab", [C, TABW], tdt).ap()

    with tile.TileContext(nc) as tc:
        nc.gpsimd.load_library(library_config.mlp)
        with (
            tc.tile_pool(name="tabp", bufs=2) as tabp,
            tc.tile_pool(name="small", bufs=2) as small,
            tc.tile_pool(name="xs", bufs=xs_bufs) as xs,
            tc.tile_pool(name="gbp", bufs=gs_bufs) as gbp,
            tc.tile_pool(name="gcp", bufs=2) as gcp,
            tc.tile_pool(name="scr", bufs=scr_bufs) as scr,
            tc.tile_pool(name="cols", bufs=1) as cols,
        ):
            # wrapped+replicated target indices for dma_gather
            idx = cols.tile([P, IDXW], i16)
            nc.sync.dma_start(idx[:], tg_ap[:, :])

            # ---- smoothing table -------------------------------------------
            for k in range((C + P - 1) // P):
                r0 = k * P
                pr = min(r0 + P, C) - r0
                cat = tabp.tile([P, C], f32, tag="cat")
                nc.sync.dma_start(cat[:pr], ca_ap[r0 : r0 + pr, :])
                e = tabp.tile([P, C], f32, tag="e")
                sume = small.tile([P, 1], f32, tag="sume")
                nc.scalar.activation(e[:pr], cat[:pr], Act.Exp, accum_out=sume[:pr])
                # diagonal e[t, t] via affine mask + row reduce
                msk = tabp.tile([P, C], f32, tag="msk")
                nc.gpsimd.affine_select(
                    out=msk[:pr], in_=e[:pr], compare_op=Alu.is_equal, fill=0.0,
                    base=-r0, channel_multiplier=-1, pattern=[[1, C]],
                )
                ett = small.tile([P, 1], f32, tag="ett")
                nc.vector.tensor_reduce(
                    out=ett[:pr], in_=msk[:pr], axis=mybir.AxisListType.X, op=Alu.add
                )
                den = small.tile([P, 1], f32, tag="den")
                nc.vector.tensor_tensor(
                    out=den[:pr], in0=sume[:pr], in1=ett[:pr], op=Alu.subtract
                )
                rec = small.tile([P, 1], f32, tag="rec")
                nc.vector.reciprocal(rec[:pr], den[:pr])
                negw = small.tile([P, 1], f32, tag="negw")
                nc.vector.tensor_scalar_mul(negw[:pr], rec[:pr], -SM)
                # S_pre[t, j] = (e - sume) * (-s / den); its diagonal equals s,
                # and sum_j S_pre = R - (1 - 2s)
                spre = tabp.tile([P, C], f32, tag="spre")
                rpre = small.tile([P, 1], f32, tag="rpre")
                nc.vector.scalar_tensor_tensor(
                    out=spre[:pr], in0=e[:pr], scalar=sume[:pr],
                    in1=negw[:pr].to_broadcast([pr, C]),
                    op0=Alu.subtract, op1=Alu.mult, accum_out=rpre[:pr],
                )
                sb = tabp.tile([P, TABW], tdt, tag="sb")
                nc.gpsimd.affine_select(
                    out=sb[:pr, 0:C], in_=spre[:pr], compare_op=Alu.not_equal,
                    fill=1.0 - SM, base=-r0, channel_multiplier=-1, pattern=[[1, C]],
                )
                # R as raw f32 bits in the 4 fp8 slots after the S entries
                rt = small.tile([P, 1], f32, tag="rt")
                nc.vector.tensor_scalar_add(rt[:pr], rpre[:pr], 1.0 - 2 * SM)
                rv = sb[:pr, C : C + rslots].bitcast(f32)
                nc.vector.tensor_copy(out=rv[:, 0:1], in_=rt[:pr])
                nc.vector.memset(sb[:pr, C + rslots : TABW], 0.0)
                nc.sync.dma_start(tab_ap[r0 : r0 + pr, :], sb[:pr])

            # ---- main loop -------------------------------------------------
            # x viewed column-major: sample j*128 + p -> partition p, col j,
            # so an x_chunk load is one DMA over a contiguous DRAM range
            x_r = x_ap.rearrange("(c p) d -> p c d", p=P)
            se_cols = cols.tile([P, NT], f32)
            dot_cols = cols.tile([P, NT], f32)
            r_cols = cols.tile([P, NT], f32)
            if abl:
                nc.vector.memset(se_cols[:], 1.0)
                nc.vector.memset(dot_cols[:], 1.0)
                nc.vector.memset(r_cols[:], 1.0)
            xt0 = gt0 = None
            if "x" in abl:
                xt0 = cols.tile([P, C], bf16)
                nc.sync.dma_start(xt0[:], x_r[:, 0, :])
            if "gather" in abl:
                gt0 = cols.tile([P, 1, TABW], tdt)
                nc.vector.memset(gt0[:, 0, :], 0.25)
            # x chunk start -> size; taper shrinks the trailing chunks so the
            # end-of-pass drain (last chunk's compute + tail with DMA idle)
            # covers 1 column instead of x_chunk
            sizes = [x_chunk] * (NT // x_chunk)
            if taper and x_chunk == 4:
                sizes = sizes[:-2] + [2, 2, 2, 1, 1]
            chunk_at = {}
            j0 = 0
            for sz in sizes:
                chunk_at[j0] = sz
                j0 += sz
            assert j0 == NT
            xbig = gband = gclean = None
            x0 = 0
            loop = tc.For_i(0, reps) if reps > 1 else contextlib.nullcontext()
            with loop:
              for j in range(NT * (unroll if reps > 1 else 1)):
                j = j % NT
                if "x" in abl:
                    xt = xt0
                else:
                    if j in chunk_at:
                        sz = chunk_at[j]
                        # uniform max-size tile so the pool ring stays regular
                        xbig = xs.tile([P, x_chunk, C], bf16)
                        nc.sync.dma_start(xbig[:, 0:sz, :], x_r[:, j : j + sz, :])
                        x0 = j
                    xt = xbig[:, j - x0, :]
                in_pure = j < pb * BCOL
                new_band = in_pure and j % BCOL == 0
                if "gather" in abl:
                    gt = gt0[:, 0, :]
                elif in_pure:
                    b = j // BCOL
                    if merge_bands:
                        if j == 0:
                            gband = gbp.tile([P, pb, TABW], tdt, tag="gband")
                            nc.gpsimd.dma_gather(
                                gband[:], tab_ap[:],
                                idx[:, 0 : pb * (P // 16)],
                                pb * P, pb * P, TABW,
                            )
                        gt = gband[:, b, :]
                    else:
                        if new_band:
                            # one row per (partition, band) cell, reused 8 cols
                            gband = gbp.tile([P, 1, TABW], tdt, tag="gband")
                            nc.gpsimd.dma_gather(
                                gband[:], tab_ap[:],
                                idx[:, b * (P // 16) : (b + 1) * (P // 16)],
                                P, P, TABW,
                            )
                        gt = gband[:, 0, :]
                else:
                    jc = j - pb * BCOL
                    if jc % BCOL == 0:
                        # per-sample gather, one 8-column chunk at a time
                        gclean = gcp.tile([P, BCOL, TABW], tdt, tag="gc")
                        c0 = pb * (P // 16) + jc * (P // 16)
                        nc.gpsimd.dma_gather(
                            gclean[:], tab_ap[:],
                            idx[:, c0 : c0 + BCOL * (P // 16)],
                            BCOL * P, BCOL * P, TABW,
                        )
                    gt = gclean[:, jc % BCOL, :]
                if "act" not in abl:
                    es = scr.tile([P, C], bf16, tag="es")
                    nc.scalar.activation(
                        es[:], xt[:], Act.Exp, accum_out=se_cols[:, j : j + 1]
                    )
                if "dve" not in abl:
                    ps = scr.tile([P, C], f32, tag="ps")
                    nc.vector.scalar_tensor_tensor(
                        out=ps[:], in0=xt[:], scalar=1.0, in1=gt[:, 0:C],
                        op0=Alu.mult, op1=Alu.mult, accum_out=dot_cols[:, j : j + 1],
                    )
                    grv = gt[:, C : C + rslots].bitcast(f32)
                    if new_band:
                        nc.vector.tensor_copy(
                            out=r_cols[:, j : j + BCOL],
                            in_=grv[:, 0:1].to_broadcast([P, BCOL]),
                        )
                    elif not in_pure:
                        nc.vector.tensor_copy(
                            out=r_cols[:, j : j + 1], in_=grv[:, 0:1]
                        )

            # ---- tail ------------------------------------------------------
            lse = cols.tile([P, NT], f32)
            nc.scalar.activation(lse[:], se_cols[:], Act.Ln)
            t1 = cols.tile([P, NT], f32)
            nc.vector.tensor_mul(t1[:], r_cols[:], lse[:])
            loss = cols.tile([P, NT], f32)
            nc.vector.tensor_tensor(
                out=loss[:], in0=t1[:], in1=dot_cols[:], op=Alu.subtract
            )
            nc.sync.dma_start(out_ap[:], loss[:])

    nc.compile()
    nc.finalize()
    return nc


def get_program(pb=PB):
    key = ("nc", pb)
    if key not in _CACHE:
        _CACHE[key] = build_program(pb=pb)
    return _CACHE[key]


def wrap_idx(stream, pb=PB):
    """[NIDX] int -> [128, NIDX/16] int16 wrapped (pos i -> [i%16, i//16])
    and replicated across the 8 partition groups of 16 (one per Q7 core)."""
    w = stream.astype(np.int16).reshape(idx_w(pb), 16).T  # [16, IDXW]
    return np.ascontiguousarray(np.tile(w, (8, 1)))  # [128, IDXW]


def max_pure_bands(ts):
    """Max pure bands a shard supports: full 8-run count // 128, capped."""
    cnt = np.bincount(ts, minlength=C)
    return int(min(PB, (cnt // BCOL).sum() // P))


def plan_shard(ts, pb=PB):
    """Pack one class-sorted shard into the slot grid.

    Returns (order, idx_stream): order[slot_linear] = shard sample row for
    device slot (p, j) with slot_linear = j*128 + p; idx_stream[n_idx(pb)] =
    the gather-index stream (pb*128 band cells then cleanup samples).
    """
    n = ts.shape[0]
    assert n == BLOC
    srt = np.argsort(ts, kind="stable")
    tss = ts[srt]
    # class group boundaries in the sorted shard
    bounds = np.flatnonzero(np.r_[True, tss[1:] != tss[:-1], True])
    runs = []          # full 8-sample runs: (class, 8 sample rows)
    cleanup = []       # leftover sample rows (class-sorted order)
    ncells = pb * P
    for g0, g1 in zip(bounds[:-1], bounds[1:]):
        cls = int(tss[g0])
        k = g0
        while k + BCOL <= g1 and len(runs) < ncells:
            runs.append((cls, srt[k : k + BCOL]))
            k += BCOL
        if k < g1:
            cleanup.append((cls, srt[k:g1]))
    assert len(runs) == ncells, (
        f"class-run packing infeasible: {len(runs)} full runs < {ncells}"
    )
    order = np.empty(n, dtype=np.int64)
    idx_stream = np.empty(n_idx(pb), dtype=np.int64)
    if ncells:
        # run r -> band b = r // 128, partition p = r % 128, cols b*8..b*8+7
        rcls = np.array([c for c, _ in runs])
        rsmp = np.stack([s for _, s in runs])          # [ncells, 8]
        r = np.arange(ncells)
        lin = ((r[:, None] // P) * BCOL + np.arange(BCOL)[None, :]) * P + (
            r[:, None] % P
        )
        order[lin.ravel()] = rsmp.ravel()
        idx_stream[:ncells] = rcls
    # cleanup sample m -> column pb*8 + m//128, partition m%128
    csmp = np.concatenate([s for _, s in cleanup]) if cleanup else np.empty(0, int)
    ccls = (
        np.concatenate([np.full(len(s), c) for c, s in cleanup])
        if cleanup
        else np.empty(0, int)
    )
    assert csmp.shape[0] == n_clean(pb), csmp.shape
    order[ncells * BCOL :] = csmp
    idx_stream[ncells:] = ccls
    return order, idx_stream


def make_in_maps(x, class_avg, target, pb=PB):
    import ml_dtypes

    x = np.ascontiguousarray(np.asarray(x, dtype=np.float32))
    ca = np.ascontiguousarray(np.asarray(class_avg, dtype=np.float32))
    tg = np.asarray(target).astype(np.int32)
    assert x.shape == (B, C) and ca.shape == (C, C) and tg.shape == (B,)
    # global class sort -> contiguous shards span ~C/8 classes each
    gsort = np.argsort(tg, kind="stable")
    maps = []
    for c in range(NCORES):
        rows = gsort[c * BLOC : (c + 1) * BLOC]
        xs, ts = x[rows], tg[rows]
        order, idx_stream = plan_shard(ts, pb)
        maps.append(
            {
                "x": np.ascontiguousarray(
                    xs[order].astype(ml_dtypes.bfloat16)
                ),
                "ca": ca,
                "tgw": wrap_idx(idx_stream, pb),
            }
        )
    return maps


def pick_pb(target):
    """Largest pb all shards support (7 for any near-uniform targets)."""
    tg = np.asarray(target).astype(np.int32)
    gsort = np.argsort(tg, kind="stable")
    return min(
        max_pure_bands(tg[gsort[c * BLOC : (c + 1) * BLOC]]) for c in range(NCORES)
    )


def reduce_outputs(results):
    tot = 0.0
    for c in range(NCORES):
        tot += results[c]["out"].astype(np.float64).sum()
    return np.array(tot / B, dtype=np.float32)


def kernel(x, class_avg, target):
    pb = pick_pb(target)
    nc = get_program(pb)
    in_maps = make_in_maps(x, class_avg, target, pb)
    res = run_bass_kernel_spmd(nc, in_maps, list(range(NCORES)))
    return reduce_outputs(res.results)
